# revision 1
# baseline (speedup 1.0000x reference)
"""Trainium2 Bass kernel: per-point 3x3 Gaussian covariance from quaternion + log_scale.

cov = R diag(exp(log_scale)) R^T  with R built from the normalized quaternion.

Layout (per core): points sharded [128 partitions, R rows]; tiles of F points
per partition; all DMAs per-partition contiguous.  Normalization folded via
inv2 = 2/|q|^2 (computed fp32 as exp(-ln(n2/2))); the multiply-heavy chain
(products -> R -> M -> Gram) runs in bf16 with contiguous step-1 operands so
VectorE hits its 2x perf mode; ScalarE does the strided deinterleave/cast,
squares, exp/ln, and output interleave.
"""

import os
import numpy as np

import concourse.bass as bass
import concourse.bacc as bacc
import concourse.mybir as mybir
from concourse.tile import TileContext
from concourse.bass_utils import run_bass_kernel_spmd

AF = mybir.ActivationFunctionType
FP32 = mybir.dt.float32
BF16 = mybir.dt.bfloat16

N_CORES = 8
N_FULL = 4_000_000
P = 128
R = 3908                      # rows per partition per core; 128*3908*8 = 4_001_792 >= N
NPC = P * R                   # points per core (padded)
F = int(os.environ.get("KERNEL_F", "448"))  # points per partition per tile

SQRT_HALF = 0.7071067811865476

_built = {}


def _build():
    key = F
    if key in _built:
        return _built[key]

    nc = bacc.Bacc("TRN2", target_bir_lowering=False, debug=False, num_devices=N_CORES)
    q = nc.dram_tensor("q", [NPC, 4], FP32, kind="ExternalInput")
    ls = nc.dram_tensor("ls", [NPC, 3], FP32, kind="ExternalInput")
    cov = nc.dram_tensor("cov", [NPC, 3, 3], FP32, kind="ExternalOutput")

    qv = q.ap().rearrange("(p r) c -> p (r c)", p=P)       # [128, 4R]
    lsv = ls.ap().rearrange("(p r) c -> p (r c)", p=P)     # [128, 3R]
    ov = cov.ap().rearrange("(p r) i k -> p (r i k)", p=P)  # [128, 9R]

    with TileContext(nc) as tc:
        with (
            tc.tile_pool(name="io", bufs=2) as io,
            tc.tile_pool(name="otp", bufs=2) as ot_pool,
            tc.tile_pool(name="big", bufs=2) as big,
            tc.tile_pool(name="wk", bufs=2) as wk,
        ):
            t0 = 0
            while t0 < R:
                f = min(F, R - t0)
                _tile_body(nc, io, ot_pool, big, wk, qv, lsv, ov, t0, f)
                t0 += f

    nc.compile()
    _built[key] = nc
    return nc


def _tile_body(nc, io, ot_pool, big, wk, qv, lsv, ov, t0, f):
    cnt = [0]

    def w(dt=BF16, tag=None):
        cnt[0] += 1
        tag = tag or f"w{cnt[0]}"
        return wk.tile([P, f], dt, tag=tag, name=f"{tag}_t{t0}_{cnt[0]}")

    qt = io.tile([P, 4 * f], FP32, tag="qt", name=f"qt{t0}")
    lst = io.tile([P, 3 * f], FP32, tag="lst", name=f"lst{t0}")
    nc.sync.dma_start(out=qt, in_=qv[:, 4 * t0:4 * (t0 + f)])
    nc.sync.dma_start(out=lst, in_=lsv[:, 3 * t0:3 * (t0 + f)])

    qc = qt.rearrange("p (f c) -> p f c", c=4)
    lsc = lst.rearrange("p (f c) -> p f c", c=3)

    # ---- fp32 path: n2/2 and inv2 = 2/|q|^2 = exp(-ln(n2/2)) -------------
    sq4 = big.tile([P, 4 * f], FP32, tag="sq4", name=f"sq4_{t0}")
    nc.scalar.activation(sq4, qt, AF.Square, scale=SQRT_HALF)  # x^2/2
    sqc = sq4.rearrange("p (f c) -> p f c", c=4)
    u = w(FP32, tag="fu"); v = w(FP32, tag="fv"); n2h = w(FP32, tag="fn2h")
    lnv = w(FP32, tag="fu"); inv2 = w(FP32, tag="fv")
    nc.vector.tensor_add(u, sqc[:, :, 0], sqc[:, :, 1])
    nc.vector.tensor_add(v, sqc[:, :, 2], sqc[:, :, 3])
    nc.vector.tensor_add(n2h, u, v)
    nc.scalar.activation(lnv, n2h, AF.Ln)
    nc.scalar.activation(inv2, lnv, AF.Exp, scale=-1.0)

    # ---- deinterleave + cast to bf16 (ScalarE, strided reads) ------------
    a_ = w(); b_ = w(); c_ = w(); d_ = w(); ivb = w()
    nc.scalar.copy(out=a_, in_=qc[:, :, 0])
    nc.scalar.copy(out=b_, in_=qc[:, :, 1])
    nc.scalar.copy(out=c_, in_=qc[:, :, 2])
    nc.scalar.copy(out=d_, in_=qc[:, :, 3])
    nc.scalar.copy(out=ivb, in_=inv2)

    # ---- bf16 chain: A..D, products (VectorE 2x mode) --------------------
    A = w(); B = w(); C = w(); D = w()
    nc.vector.tensor_mul(A, ivb, a_)
    nc.vector.tensor_mul(B, ivb, b_)
    nc.vector.tensor_mul(C, ivb, c_)
    nc.vector.tensor_mul(D, ivb, d_)

    Ab = w(); Ac = w(); Ad = w()
    Bb = w(); Bc = w(); Bd = w()
    Cc = w(); Cd = w(); Dd = w()
    nc.vector.tensor_mul(Ab, A, b_)
    nc.vector.tensor_mul(Ac, A, c_)
    nc.vector.tensor_mul(Ad, A, d_)
    nc.vector.tensor_mul(Bb, B, b_)
    nc.vector.tensor_mul(Bc, B, c_)
    nc.vector.tensor_mul(Bd, B, d_)
    nc.vector.tensor_mul(Cc, C, c_)
    nc.vector.tensor_mul(Cd, C, d_)
    nc.vector.tensor_mul(Dd, D, d_)

    # ---- rotation matrix entries (bf16) ----------------------------------
    t_0 = w(); t_1 = w(); t_2 = w()
    nc.vector.tensor_add(t_0, Cc, Dd)
    nc.vector.tensor_add(t_1, Bb, Dd)
    nc.vector.tensor_add(t_2, Bb, Cc)
    r00 = w(FP32, tag="fr00"); r11 = w(FP32, tag="fr11"); r22 = w(FP32, tag="fr22")
    nc.scalar.activation(r00, t_0, AF.Identity, bias=1.0, scale=-1.0)
    nc.scalar.activation(r11, t_1, AF.Identity, bias=1.0, scale=-1.0)
    nc.scalar.activation(r22, t_2, AF.Identity, bias=1.0, scale=-1.0)
    r01 = w(); r10 = w(); r02 = w(); r20 = w(); r12 = w(); r21 = w()
    nc.vector.tensor_sub(r01, Bc, Ad)
    nc.vector.tensor_add(r10, Bc, Ad)
    nc.vector.tensor_add(r02, Bd, Ac)
    nc.vector.tensor_sub(r20, Bd, Ac)
    nc.vector.tensor_sub(r12, Cd, Ab)
    nc.vector.tensor_add(r21, Cd, Ab)

    # ---- sqrt(scale) per column (ScalarE, bf16 contiguous out) -----------
    sh = [w(FP32, tag="fsh0"), w(FP32, tag="fsh1"), w(FP32, tag="fsh2")]
    for j in range(3):
        nc.scalar.activation(sh[j], lsc[:, :, j], AF.Exp, scale=0.5)

    Rm = [[r00, r01, r02], [r10, r11, r12], [r20, r21, r22]]
    M = [[None] * 3 for _ in range(3)]
    for i in range(3):
        for j in range(3):
            M[i][j] = w(FP32 if i == j else BF16, tag=f"pm{i}{j}")
            nc.vector.tensor_mul(M[i][j], Rm[i][j], sh[j])

    # ---- cov = M M^T; diag entries write straight into the out tile ------
    ot = ot_pool.tile([P, 9 * f], FP32, tag="ot", name=f"ot_{t0}")
    otv = ot.rearrange("p (f e) -> p f e", e=9)
    offd = {}
    for (i, k) in [(0, 0), (0, 1), (0, 2), (1, 1), (1, 2), (2, 2)]:
        fd = i == k
        g = w(FP32 if fd else BF16, tag="ggf" if fd else "gg")
        g2 = w(FP32 if fd else BF16, tag="gg2f" if fd else "gg2")
        h = w(tag="gh"); h2 = w(tag="gh2")
        nc.vector.tensor_mul(g, M[i][0], M[k][0])
        nc.vector.tensor_mul(h, M[i][1], M[k][1])
        nc.vector.tensor_add(g2, g, h)
        nc.vector.tensor_mul(h2, M[i][2], M[k][2])
        if i == k:
            nc.vector.tensor_add(otv[:, :, 3 * i + k], g2, h2)  # fp32 strided out
        else:
            cik = w(tag=f"cov{i}{k}")
            nc.vector.tensor_add(cik, g2, h2)
            offd[(i, k)] = cik

    # off-diagonals + symmetric duplicates via ScalarE copies (cast to fp32)
    for (i, k), cik in offd.items():
        nc.scalar.copy(out=otv[:, :, 3 * i + k], in_=cik)
        nc.scalar.copy(out=otv[:, :, 3 * k + i], in_=cik)

    nc.sync.dma_start(out=ov[:, 9 * t0:9 * (t0 + f)], in_=ot)


def _pad_and_shard(quaternion, log_scale):
    n = quaternion.shape[0]
    pad = N_CORES * NPC - n
    if pad:
        qpad = np.tile(np.array([1, 0, 0, 0], np.float32), (pad, 1))
        lpad = np.zeros((pad, 3), np.float32)
        quaternion = np.concatenate([quaternion, qpad], axis=0)
        log_scale = np.concatenate([log_scale, lpad], axis=0)
    in_maps = []
    for i in range(N_CORES):
        sl = slice(i * NPC, (i + 1) * NPC)
        in_maps.append({
            "q": np.ascontiguousarray(quaternion[sl]),
            "ls": np.ascontiguousarray(log_scale[sl]),
        })
    return in_maps


def kernel_with_stats(quaternion, log_scale, trace=False):
    quaternion = np.asarray(quaternion, dtype=np.float32)
    log_scale = np.asarray(log_scale, dtype=np.float32)
    n = quaternion.shape[0]
    nc = _build()
    in_maps = _pad_and_shard(quaternion, log_scale)
    res = run_bass_kernel_spmd(nc, in_maps, core_ids=list(range(N_CORES)), trace=trace)
    out = np.concatenate([r["cov"] for r in res.results], axis=0)[:n]
    return out, res


def kernel(quaternion, log_scale):
    out, _ = kernel_with_stats(quaternion, log_scale, trace=False)
    return out



# revision 4
# speedup vs baseline: 2.2252x; 2.2252x over previous
"""Trainium2 Bass kernel: per-point 3x3 Gaussian covariance from quaternion + log_scale.

cov = R diag(exp(log_scale)) R^T with R from the normalized quaternion.

Identity used (avoids normalizing q and the third rotation column):
  nu  = |q|^2 / 2
  C   = nu * R   (entries are plain quadratics of raw q: C00 = ha+hb-hc-hd,
                  C10 = bc+ad, C20 = bd-ac, C01 = bc-ad, C11 = ha-hb+hc-hd,
                  C21 = cd+ab, with hx = x^2/2)
  cov = s2*I + t0'*C0 C0^T + t1'*C1 C1^T,  tj' = (sj - s2)/nu^2
(uses R R^T = I to eliminate column 2.)

Layout: host uploads planar fp16 q[4, NPC], ls[3, NPC]; output is the 6
unique covariance entries planar fp16 cov6[6, NPC]; host symmetrizes and
upcasts.  On-chip everything is fp16 contiguous/block APs so every DVE
tensor_tensor hits the 2x_1p mode; squares/exp/copy run on ScalarE (all in
the one `exp_and_others` table set -> single ACT_TABLE_LOAD).  1/nu^2 via
the custom DVE reciprocal_approx_fast (no Ln, no table switch).
"""

import os
import numpy as np

import concourse.bass as bass
import concourse.bacc as bacc
import concourse.mybir as mybir
from concourse.tile import TileContext
from concourse.bass_utils import run_bass_kernel_spmd

AF = mybir.ActivationFunctionType
FP32 = mybir.dt.float32
FP16 = mybir.dt.float16

N_CORES = 8
N_FULL = 4_000_000
P = 128

F = int(os.environ.get("KERNEL_F", "977"))      # points per partition per tile
NT = -(-3907 // F)                               # tiles so that P*R*8 >= N
R = F * NT                                       # rows per partition per core
NPC = P * R                                      # points per core (padded)

SQRT_HALF = 0.7071067811865476

_built = {}


def _apv(t, off, pairs):
    """Raw AP view of tile t: keep its partition dim, replace free dims.

    pairs = [[stride, count], ...] in elements, offset in elements from the
    tile's base.
    """
    ap = [list(p) for p in t.ap]
    return bass.AP(tensor=t.tensor, offset=t.offset + off, ap=[ap[0]] + pairs)


def _bc(ap2d, n):
    """[P, f] -> [P, n, f] broadcast (stride-0 middle dim)."""
    p, f = ap2d.shape
    return ap2d.unsqueeze(1).broadcast_to((p, n, f))


def _build():
    key = F
    if key in _built:
        return _built[key]

    nc = bacc.Bacc("TRN2", target_bir_lowering=False, debug=False, num_devices=N_CORES)
    q = nc.dram_tensor("q", [4, NPC], FP16, kind="ExternalInput")
    ls = nc.dram_tensor("ls", [3, NPC], FP16, kind="ExternalInput")
    cov = nc.dram_tensor("cov6", [6, NPC], FP16, kind="ExternalOutput")

    qv = q.ap().rearrange("c (p r) -> p c r", p=P)      # [P, 4, R]
    lsv = ls.ap().rearrange("c (p r) -> p c r", p=P)    # [P, 3, R]
    ov = cov.ap().rearrange("e (p r) -> p e r", p=P)    # [P, 6, R]

    with TileContext(nc) as tc:
        with (
            tc.tile_pool(name="io", bufs=2) as io,
            tc.tile_pool(name="wk", bufs=2) as wk,
        ):
            for it in range(NT):
                _tile_body(nc, io, wk, qv, lsv, ov, it * F, F)

    nc.compile()
    _built[key] = nc
    return nc


def _tile_body(nc, io, wk, qv, lsv, ov, t0, f):
    v = nc.vector
    s = nc.scalar

    def W(shape_f, dt=FP16, tag=None):
        return wk.tile([P, shape_f], dt, tag=tag, name=f"{tag}_{t0}")

    # ---- DMA in ----------------------------------------------------------
    q4 = io.tile([P, 4 * f], FP16, tag="q4", name=f"q4_{t0}")
    ls3 = io.tile([P, 3 * f], FP16, tag="ls3", name=f"ls3_{t0}")
    nc.sync.dma_start(out=q4.rearrange("p (c x) -> p c x", c=4),
                      in_=qv[:, :, t0:t0 + f])
    nc.sync.dma_start(out=ls3.rearrange("p (c x) -> p c x", c=3),
                      in_=lsv[:, :, t0:t0 + f])

    # ---- half-squares + nu + diagonal R entries --------------------------
    sq4 = W(4 * f, tag="sq4_pa")            # [ha|hb|hc|hd]
    s.activation(sq4, q4, AF.Square, scale=SQRT_HALF)

    pq = W(2 * f, tag="pq_pb")              # [ha-hc | hb-hd]
    st = W(2 * f, tag="st_tt")              # [ha+hc | hb+hd]
    v.tensor_sub(pq, sq4[:, 0:2 * f], sq4[:, 2 * f:4 * f])
    v.tensor_add(st, sq4[:, 0:2 * f], sq4[:, 2 * f:4 * f])

    r6 = W(6 * f, tag="r6")                 # [C00|C10|C20|C01|C11|C21]
    v.tensor_add(r6[:, 0:f], pq[:, 0:f], pq[:, f:2 * f])          # C00
    v.tensor_sub(r6[:, 4 * f:5 * f], pq[:, 0:f], pq[:, f:2 * f])  # C11
    nu = W(f, FP32, tag="nu")
    v.tensor_add(nu, st[:, 0:f], st[:, f:2 * f])

    # ---- raw quaternion products -----------------------------------------
    pp = W(3 * f, tag="pp")                 # [ab|ac|ad]
    qcd = W(3 * f, tag="qcd")               # [bc|bd|cd]
    q4c = q4.rearrange("p (c x) -> p c x", c=4)
    v.tensor_mul(pp.rearrange("p (c x) -> p c x", c=3),
                 _bc(q4[:, 0:f], 3), q4c[:, 1:4, :])
    v.tensor_mul(qcd.rearrange("p (c x) -> p c x", c=3)[:, 0:2, :],
                 _bc(q4[:, f:2 * f], 2), q4c[:, 2:4, :])
    v.tensor_mul(qcd[:, 2 * f:3 * f], q4[:, 2 * f:3 * f], q4[:, 3 * f:4 * f])

    # ---- off-diagonal R entries (paired block ops) ------------------------
    # [C10|C21] = [bc|cd] + [ad|ab]
    v.tensor_add(_apv(r6, f, [[4 * f, 2], [1, f]]),
                 _apv(qcd, 0, [[2 * f, 2], [1, f]]),
                 _apv(pp, 2 * f, [[-2 * f, 2], [1, f]]))
    # [C20|C01] = [bd|bc] - [ac|ad]
    v.tensor_sub(_apv(r6, 2 * f, [[f, 2], [1, f]]),
                 _apv(qcd, f, [[-f, 2], [1, f]]),
                 _apv(pp, f, [[f, 2], [1, f]]))

    # ---- scales: s3 = exp(ls), tt = (sj - s2) / nu^2 ----------------------
    s3 = W(3 * f, tag="s3")
    s.activation(s3, ls3, AF.Exp)
    s2v = s3[:, 2 * f:3 * f]

    nusq = W(f, FP32, tag="nusq")
    s.activation(nusq, nu, AF.Square)
    iv = W(f, FP32, tag="iv")
    v.reciprocal_approx_fast(iv, nusq)
    ivh = W(f, FP16, tag="ivh")
    s.copy(out=ivh, in_=iv)

    tt = st                                  # reuse [P, 2f] (st dead)
    ttj = tt.rearrange("p (j x) -> p j x", j=2)
    v.tensor_sub(ttj,
                 s3.rearrange("p (j x) -> p j x", j=3)[:, 0:2, :],
                 _bc(s2v, 2))
    v.tensor_mul(ttj, ttj, _bc(ivh, 2))

    # ---- Gram: cov6 = [c00|c11|c22|c01|c02|c12] ---------------------------
    sq6 = W(6 * f, tag="sq6")                # C entries squared (ScalarE)
    s.activation(sq6, r6, AF.Square)
    # wsq = sq6 * t_j  (in place over sq6)
    v.tensor_mul(sq6.rearrange("p (j i x) -> p j i x", j=2, i=3),
                 sq6.rearrange("p (j i x) -> p j i x", j=2, i=3),
                 ttj.unsqueeze(2).broadcast_to((P, 2, 3, f)))

    # v4 = [v00|v10|v01|v11] = t_j * (C0j, C1j)
    v4 = W(4 * f, tag="v4")
    r6j = r6.rearrange("p (j i x) -> p j i x", j=2, i=3)
    v.tensor_mul(v4.rearrange("p (j i x) -> p j i x", j=2, i=2),
                 r6j[:, :, 0:2, :],
                 ttj.unsqueeze(2).broadcast_to((P, 2, 2, f)))

    # pa = [v00*C10 | v00*C20 | v01*C11 | v01*C21]
    pa = sq4                                 # reuse [P, 4f] (sq4 dead)
    v4j = v4.rearrange("p (j i x) -> p j i x", j=2, i=2)
    v.tensor_mul(pa.rearrange("p (j i x) -> p j i x", j=2, i=2),
                 v4j[:, :, 0:1, :].broadcast_to((P, 2, 2, f)),
                 r6j[:, :, 1:3, :])
    # pb = [v10*C20 | v11*C21]
    pb = pq                                  # reuse [P, 2f] (pq dead)
    v.tensor_mul(pb.rearrange("p (j x) -> p j x", j=2),
                 v4j[:, :, 1:2, :].squeeze(2),
                 r6j[:, :, 2:3, :].squeeze(2))

    out6 = io.tile([P, 6 * f], FP16, tag="out6", name=f"out6_{t0}")
    # off-diagonals
    v.tensor_add(out6[:, 3 * f:5 * f], pa[:, 0:2 * f], pa[:, 2 * f:4 * f])
    v.tensor_add(out6[:, 5 * f:6 * f], pb[:, 0:f], pb[:, f:2 * f])
    # diagonal: wsq[0:3]+wsq[3:6] + s2
    v.tensor_add(sq6[:, 0:3 * f], sq6[:, 0:3 * f], sq6[:, 3 * f:6 * f])
    v.tensor_add(out6.rearrange("p (e x) -> p e x", e=6)[:, 0:3, :],
                 sq6.rearrange("p (e x) -> p e x", e=6)[:, 0:3, :],
                 _bc(s2v, 3))

    nc.sync.dma_start(out=ov[:, :, t0:t0 + f],
                      in_=out6.rearrange("p (e x) -> p e x", e=6))


def _prep_inputs(quaternion, log_scale):
    n = quaternion.shape[0]
    pad = N_CORES * NPC - n
    if pad:
        qpad = np.tile(np.array([1, 0, 0, 0], np.float32), (pad, 1))
        lpad = np.zeros((pad, 3), np.float32)
        quaternion = np.concatenate([quaternion, qpad], axis=0)
        log_scale = np.concatenate([log_scale, lpad], axis=0)
    in_maps = []
    for i in range(N_CORES):
        sl = slice(i * NPC, (i + 1) * NPC)
        in_maps.append({
            "q": np.ascontiguousarray(quaternion[sl].T.astype(np.float16)),
            "ls": np.ascontiguousarray(log_scale[sl].T.astype(np.float16)),
        })
    return in_maps


def kernel_with_stats(quaternion, log_scale, trace=False):
    quaternion = np.asarray(quaternion, dtype=np.float32)
    log_scale = np.asarray(log_scale, dtype=np.float32)
    n = quaternion.shape[0]
    nc = _build()
    in_maps = _prep_inputs(quaternion, log_scale)
    res = run_bass_kernel_spmd(nc, in_maps, core_ids=list(range(N_CORES)), trace=trace)
    planes = np.concatenate([r["cov6"] for r in res.results], axis=1)[:, :n]
    planes = planes.astype(np.float32)
    out = np.empty((n, 3, 3), np.float32)
    out[:, 0, 0] = planes[0]
    out[:, 1, 1] = planes[1]
    out[:, 2, 2] = planes[2]
    out[:, 0, 1] = out[:, 1, 0] = planes[3]
    out[:, 0, 2] = out[:, 2, 0] = planes[4]
    out[:, 1, 2] = out[:, 2, 1] = planes[5]
    return out, res


def kernel(quaternion, log_scale):
    out, _ = kernel_with_stats(quaternion, log_scale, trace=False)
    return out
